# revision 34
# baseline (speedup 1.0000x reference)
"""Trainium2 Bass kernel for nn_BinaryConnectNet (binary CNN, 8 NeuronCores).

Sharding: batch-parallel convs (128 img/core), fc1 output-feature-sharded
(128 features/core) with an on-device AllGather of the binary activations
(fp8, values +-1). fc2 computed as per-core partials, summed on host.

Numerics:
 - conv1 (dw3x3 + 1x1 fused into a dense 3x3 with +-1 weights): input x is
   triple-bf16-split (exact to 2^-24 -- conv1 sign flips cascade through the
   binary net, so conv1 must be exact) contracted in one K=81 matmul per
   512-col quadrant chunk. All 4 pool quadrants accumulate into one
   [128,2048] PSUM tile; eviction is 3 ops: ACT sign(+bias) -> DVE dy-max ->
   GPSIMD dx-max writing +-1 fp8 into the padded h1 buffer.
 - conv2 depthwise: 9 PSUM-accumulated diagonal fp8 matmuls over shifted
   windows of h1. dw bias is folded into the pw bias (no nonlinearity
   between them), so dw eviction is a plain copy (odd ints |v|<=9, exact
   in fp8).
 - conv2 pointwise: dense K=128 fp8 matmul; sign bias = sgn(b_pw) +
   s_pw @ sgn(b_dw) (odd, so the pre-sign value is never 0).
 - fc1: weights split hi/lo fp16 (exact to 2^-22) accumulated into one PSUM;
   rhs is the gathered +-1 activations in fp8. The hi half is prefetched
   into a persistent SBUF tensor during the AllGather.
 - fc2: fp16 hi/lo, per-core partial output in fp32.
"""

import sys

for _p in ("/opt/trn_rl_repo",):
    if _p not in sys.path:
        sys.path.insert(0, _p)

import numpy as np
import ml_dtypes
from contextlib import ExitStack

import concourse.bass as bass
import concourse.bacc as bacc
import concourse.mybir as mybir
import concourse.tile as tile
from concourse.bass_utils import run_bass_kernel_spmd

F32 = mybir.dt.float32
BF16 = mybir.dt.bfloat16
FP16 = mybir.dt.float16
FP8 = mybir.dt.float8e4
AF = mybir.ActivationFunctionType
ALU = mybir.AluOpType

NCORES = 8
C1_SWDGE_POOL = False  # conv1 pooling via gpsimd accum-DMA (else DVE maxes)
B = 128               # images per core
H = 32                # conv1 spatial
HP = 34               # padded
ROWLEN = B * HP       # 4352: one padded h-row across batch (b, w) flattened
X9_SLACK = 8
X9_ROW = HP * ROWLEN + X9_SLACK   # flattened (h, b, w) per (c, s) row + slack
P1 = 16               # pooled spatial after pool1
P1PAD = 18
P2 = 8                # pooled spatial after pool2
NF1 = 1024            # fc1 features (global)
FPC = NF1 // NCORES   # fc1 features per core = 128
KFC = 256 * P2 * P2   # fc1 contraction = 16384
NKT = KFC // 128      # 128 K-tiles
NB_ALL = NCORES * B   # 1024


def _bf16(a):
    return np.asarray(a, dtype=ml_dtypes.bfloat16)


def _host_prep(x, w1_dw, b1_dw, w1_pw, b1_pw, w2_dw, b2_dw, w2_pw, b2_pw,
               fc1_w, fc1_b, fc2_w, fc2_b, ncores=NCORES, nb=B):
    """Build all per-core device input arrays (numpy only)."""
    sgn = np.sign
    x = np.asarray(x, np.float32)
    rowlen = nb * HP
    x9row = HP * rowlen + X9_SLACK
    fpc = FPC

    # triple bf16 split of x (exact to 2^-24)
    x0 = _bf16(x)
    r1 = x - x0.astype(np.float32)
    x1 = _bf16(r1)
    r2 = r1 - x1.astype(np.float32)
    x2 = _bf16(r2)
    splits = [x0, x1, x2]

    # x9h: per core [9 rows (3c+s), (h, b, w) flattened] bf16, pad 1
    x9h = np.zeros((ncores, 9, x9row), dtype=ml_dtypes.bfloat16)
    for s in range(3):
        xs = splits[s].reshape(ncores, nb, 3, H, H)
        for c in range(3):
            row = np.zeros((ncores, HP, nb, HP), dtype=ml_dtypes.bfloat16)
            row[:, 1:33, :, 1:33] = xs[:, :, c].transpose(0, 2, 1, 3)
            x9h[:, 3 * c + s, : HP * rowlen] = row.reshape(ncores, -1)
    # x81: host-side im2col so each pooled row needs ONE contiguous DMA:
    # [nh2, 81 rows (du,dv,c,s), 2 h-rows (h, b, w)]
    nh2_ = H // 2
    x81 = np.zeros((ncores, nh2_, 128, 2 * rowlen), dtype=ml_dtypes.bfloat16)
    for hc in range(nh2_):
        for du in range(3):
            for dv in range(3):
                r0 = 9 * (3 * du + dv)
                off = (2 * hc + du) * rowlen + dv
                x81[:, hc, r0:r0 + 9, :] = x9h[:, :, off:off + 2 * rowlen]
    # pad K from 81 to 128 (the PE clock monitor ignores partial-K matmuls,
    # leaving the array throttled to 1.2GHz): rows 81..127 duplicate rows
    # 0..46 and both copies carry half weight (+-0.5, exact in bf16)
    x81[:, :, 81:128, :] = x81[:, :, 0:47, :]

    # conv1 fused weights: lhsT [81, 128], rows (du,dv,c,s)
    s1dw = sgn(np.asarray(w1_dw, np.float32))[:, 0]       # [3, 3, 3]
    s1pw = sgn(np.asarray(w1_pw, np.float32))[:, :, 0, 0]  # [128, 3]
    w1t = np.zeros((128, 128), dtype=ml_dtypes.bfloat16)
    for du in range(3):
        for dv in range(3):
            for c in range(3):
                for s in range(3):
                    w1t[9 * (3 * du + dv) + 3 * c + s] = _bf16(
                        s1pw[:, c] * s1dw[c, du, dv])
    w1t[81:128] = w1t[0:47]
    w1t[0:47] = w1t[0:47] * _bf16(0.5)
    w1t[81:128] = w1t[81:128] * _bf16(0.5)
    b1eff = (sgn(np.asarray(b1_pw, np.float32))
             + s1pw @ sgn(np.asarray(b1_dw, np.float32))).astype(np.float32)

    # conv2 depthwise: 9 diagonal lhsT [128, 9*128] fp8
    s2dw = sgn(np.asarray(w2_dw, np.float32))[:, 0]       # [128, 3, 3]
    dwt = np.zeros((128, 9 * 128), dtype=ml_dtypes.bfloat16)
    for du in range(3):
        for dv in range(3):
            t = 3 * du + dv
            np.fill_diagonal(dwt[:, 128 * t:128 * (t + 1)],
                             _bf16(s2dw[:, du, dv]))
    dwb = sgn(np.asarray(b2_dw, np.float32))               # [128]

    # conv2 pointwise lhsT [128, 256] fp8; bias folds in the dw bias
    s2pw = sgn(np.asarray(w2_pw, np.float32))[:, :, 0, 0]  # [256, 128]
    pwt = _bf16(s2pw.T)                                    # [128, 256]
    b2m = (sgn(np.asarray(b2_pw, np.float32)) + s2pw @ dwb).astype(np.float32)

    # fc1 hi/lo fp16, column-permuted to device K-tile order, per-core slice
    fc1_w = np.asarray(fc1_w, np.float32)                  # [1024, 16384]
    # device feature order: kt = ct*64 + s0, partition c' -> col (ct*128+c')*64+s0
    cols = np.empty(KFC, np.int64)
    i = 0
    for ct in range(2):
        for s0 in range(64):
            for cp in range(128):
                cols[i] = (ct * 128 + cp) * 64 + s0
                i += 1
    wperm = fc1_w[:, cols]                                 # [1024, 16384(dev)]
    whi = wperm.astype(np.float16)
    wlo = (wperm - whi.astype(np.float32)).astype(np.float16)
    # whi: per-core [128(c'), NKT*128(kt-major)] for the one-shot SBUF DMA
    whi_t = whi.reshape(NCORES, fpc, NKT, 128).transpose(0, 3, 2, 1) \
        .reshape(NCORES, 128, NKT * fpc).copy()
    # wlo: per-core [NKT, 128(c'), 128(o_local)] streamed per kt
    wlo_t = wlo.reshape(NCORES, fpc, NKT, 128).transpose(0, 2, 3, 1).copy()
    b1fc = np.asarray(fc1_b, np.float32).reshape(NCORES, fpc, 1)

    # fc2 hi/lo fp16 per-core slice: lhsT [128(f_local), 10]
    fc2_w = np.asarray(fc2_w, np.float32)                  # [10, 1024]
    f2 = fc2_w.T.reshape(NCORES, fpc, 10)
    f2hi = f2.astype(np.float16)
    f2lo = (f2 - f2hi.astype(np.float32)).astype(np.float16)

    shared = {
        "w1t": w1t, "b1eff": b1eff.reshape(128, 1),
        "negb1": (-b1eff).reshape(128, 1).astype(np.float32),
        "dwt": dwt,
        "pwt": pwt, "b2m": b2m.reshape(2, 128).T.copy().astype(np.float32),
        "negb2": (-b2m).reshape(2, 128).T.copy().astype(np.float32),
    }
    per_core = []
    for n in range(ncores):
        d = dict(shared)
        d["x9h"] = x81[n]
        d["whi"] = whi_t[n]
        d["wlo"] = wlo_t[n]
        d["fc1b"] = b1fc[n]
        d["f2hi"] = f2hi[n]
        d["f2lo"] = f2lo[n]
        per_core.append(d)
    return per_core


def build_program(ncores=NCORES, nb=B, repeats=1):
    """Build the Bass program. nb = images per core. Returns nc."""
    rowlen = nb * HP
    x9row = HP * rowlen + X9_SLACK
    nsh = ncores                    # shards gathered for fc1
    nball = ncores * nb             # total batch
    nbc = 2                         # fc1 batch chunks = gather halves
    bc_n = nball // nbc             # fc1 chunk width (N per matmul)

    nc = bacc.Bacc("TRN2", target_bir_lowering=False, debug=False,
                   num_devices=ncores)

    def din(name, shape, dt):
        return nc.dram_tensor(name, shape, dt, kind="ExternalInput").ap()

    x9h = din("x9h", [H // 2, 128, 2 * rowlen], BF16)
    w1t = din("w1t", [128, 128], BF16)
    b1eff = din("b1eff", [128, 1], F32)
    negb1 = din("negb1", [128, 1], F32)
    dwt = din("dwt", [128, 9 * 128], BF16)
    pwt = din("pwt", [128, 256], BF16)
    b2m = din("b2m", [128, 2], F32)
    negb2 = din("negb2", [128, 2], F32)
    whi = din("whi", [128, NKT * FPC], FP16)
    wlo = din("wlo", [NKT, 128, FPC], FP16)
    fc1b = din("fc1b", [FPC, 1], F32)
    f2hi = din("f2hi", [FPC, 10], FP16)
    f2lo = din("f2lo", [FPC, 10], FP16)
    y_out = nc.dram_tensor("y", [10, nball], F32, kind="ExternalOutput").ap()

    # collective bounce buffers (fp8 +-1 activations), split in two
    # image-halves so the first gather overlaps conv2's second half
    nbh = nb * 32                   # elements per (half, mt): 64 s x nb/2
    h2_shard = nc.dram_tensor("h2_shard", [2, 2, 128, nbh], FP8).ap()
    h2_all = nc.dram_tensor("h2_all", [2, nsh, 2, 128, nbh], FP8,
                            addr_space="Shared").ap()

    nh2 = H // 2  # 16 pooled rows after pool1

    with ExitStack() as octx:
      # persistent SBUF living across the TileContext phases
      whi_sb = octx.enter_context(
          nc.sbuf_tensor("whi_sb", [128, NKT * FPC], FP16))
      whi_ap = whi_sb.ap()
      cctx = octx.enter_context(ExitStack())  # conv-phase persistents
      h1_sb = cctx.enter_context(
          nc.sbuf_tensor("h1_sb", [128, nb * P1PAD * P1PAD], FP8))
      h2_sb = [cctx.enter_context(
          nc.sbuf_tensor(f"h2_sb{m}", [128, nb * 64], FP8))
          for m in range(2)]
      dw_sb = cctx.enter_context(
          nc.sbuf_tensor("dw_sb", [128, 9 * 128], BF16))
      pw_sb = cctx.enter_context(nc.sbuf_tensor("pw_sb", [128, 256], BF16))
      b2_sb = cctx.enter_context(nc.sbuf_tensor("b2_sb", [128, 2], F32))
      nb2_sb = cctx.enter_context(nc.sbuf_tensor("nb2_sb", [128, 2], F32))
      dw_t, pw_t = dw_sb.ap(), pw_sb.ap()
      b2_t, nb2_t = b2_sb.ap(), nb2_sb.ap()
      h1v = h1_sb.ap().rearrange("p (b y x) -> p b y x", b=nb, y=P1PAD)

      cimg = 16                     # conv2 images per chunk
      ncch = nb // cimg
      ghalf = max(1, ncch // 2)     # g-chunks per gather half
      jw = 2                        # images per dw matmul
      jw2 = 8                       # images per pw matmul

      def emit_conv2_half(tc, hf):
        """Emit conv2 for image-half hf and ship its shard slices."""
        with tc.tile_pool(name=f"dwch{hf}", bufs=2) as dwpool, \
             tc.tile_pool(name=f"pwsq{hf}", bufs=5) as sq2pool, \
             tc.tile_pool(name=f"pwtr{hf}", bufs=2) as tr2pool, \
             tc.tile_pool(name=f"dwps{hf}", bufs=2, space="PSUM") as dps, \
             tc.tile_pool(name=f"pwps{hf}", bufs=2, space="PSUM") as pps:
            for gl in range(ghalf):
                g = hf * ghalf + gl
                dwc = dwpool.tile([128, cimg * 256], BF16, tag="dwc")
                gsub = 4             # images per dw psum
                for sub in range(cimg // gsub):
                    ps = dps.tile([128, gsub * 256], F32, tag="dps")
                    for t in range(9):
                        du, dv = t // 3, t % 3
                        for j in range(gsub // jw):
                            b0 = g * cimg + sub * gsub + j * jw
                            nc.tensor.matmul(
                                ps[:, j * jw * 256:(j + 1) * jw * 256],
                                dw_t[:, 128 * t:128 * (t + 1)],
                                h1v[:, b0:b0 + jw, du:du + P1,
                                    dv:dv + P1],
                                start=(t == 0), stop=(t == 8))
                    # dw bias folded into pw bias: plain copy eviction
                    if sub % 2 == 0:
                        nc.scalar.copy(
                            dwc[:, sub * gsub * 256:(sub + 1) * gsub * 256],
                            ps[:])
                    else:
                        nc.vector.tensor_copy(
                            dwc[:, sub * gsub * 256:(sub + 1) * gsub * 256],
                            ps[:])
                dwv = dwc[:].rearrange(
                    "p (b y2 dy x2 dx) -> p b y2 dy x2 dx",
                    b=cimg, y2=P2, dy=2, x2=P2)
                for mt in range(2):
                    sq2 = []
                    for qi, (dy, dx) in enumerate(
                            ((0, 0), (0, 1), (1, 0), (1, 1))):
                        sqt = sq2pool.tile([128, cimg * 64], BF16,
                                           tag="sq2")
                        ps = pps.tile([128, cimg * 64], F32, tag="pps")
                        for j in range(cimg // jw2):
                            nc.tensor.matmul(
                                ps[:, j * jw2 * 64:(j + 1) * jw2 * 64],
                                pw_t[:, 128 * mt:128 * (mt + 1)],
                                dwv[:, j * jw2:(j + 1) * jw2,
                                    :, dy, :, dx],
                                start=True, stop=True)
                        if qi < 3:
                            nc.scalar.activation(
                                sqt[:], ps[:], AF.Sign,
                                bias=b2_t[:, mt:mt + 1])
                        else:
                            nc.vector.tensor_scalar(
                                sqt[:], ps[:], nb2_t[:, mt:mt + 1],
                                2.0, ALU.is_ge, ALU.mult)
                            nc.vector.tensor_scalar(
                                sqt[:], sqt[:], 1.0, None, ALU.subtract)
                        sq2.append(sqt)
                    u01 = tr2pool.tile([128, cimg * 64], BF16, tag="tr2")
                    u23 = tr2pool.tile([128, cimg * 64], BF16, tag="tr2")
                    nc.vector.tensor_max(u01[:], sq2[0][:], sq2[1][:])
                    nc.vector.tensor_max(u23[:], sq2[2][:], sq2[3][:])
                    # h2 layout: (half, bh, s) so shard halves are contiguous
                    h2v = h2_sb[mt].ap().rearrange(
                        "p (hf s bh) -> p hf bh s", hf=2, s=64)
                    bh0 = gl * cimg
                    nc.vector.tensor_max(
                        h2v[:, hf, bh0:bh0 + cimg, :],
                        u01[:].rearrange("p (b s) -> p b s", b=cimg),
                        u23[:].rearrange("p (b s) -> p b s", b=cimg))
        # ship this half of the shard (contiguous 16B-aligned slices)
        for mt in range(2):
            nc.scalar.dma_start(h2_shard[hf, mt],
                                h2_sb[mt].ap()[:, hf * nbh:(hf + 1) * nbh])

      for _rep in range(repeats):
        with tile.TileContext(nc) as tc, ExitStack() as ctx:
          cpool = ctx.enter_context(tc.tile_pool(name="consts", bufs=1))
          w1_t = cpool.tile([128, 128], BF16)
          nc.scalar.dma_start(w1_t[:], w1t[:])
          b1_t = cpool.tile([128, 1], F32)
          nc.scalar.dma_start(b1_t[:], b1eff[:])
          nb1_t = cpool.tile([128, 1], F32)
          nc.scalar.dma_start(nb1_t[:], negb1[:])
          nc.scalar.dma_start(dw_t[:], dwt[:])
          nc.scalar.dma_start(pw_t[:], pwt[:])
          nc.scalar.dma_start(b2_t[:], b2m[:])
          nc.scalar.dma_start(nb2_t[:], negb2[:])
          # prefetch the fc1 hi weights during conv (scalar DMA queue)
          nc.scalar.dma_start(whi_ap[:], whi[:])

          # zero the h1 pad border
          nc.vector.memset(h1v[:, :, 0, :], 0.0)
          nc.vector.memset(h1v[:, :, P1PAD - 1, :], 0.0)
          nc.vector.memset(h1v[:, :, 1:P1PAD - 1, 0], 0.0)
          nc.vector.memset(h1v[:, :, 1:P1PAD - 1, P1PAD - 1], 0.0)

          # ---- conv1 + pool1 -> h1 (padded, +-1 fp8) ----
          with tc.tile_pool(name="c1work", bufs=3) as impool, \
               tc.tile_pool(name="c1sq", bufs=4) as sqpool, \
               tc.tile_pool(name="c1m2", bufs=4) as m2pool:
              nj = (nb * 16) // 512            # 512-col chunks per hc
              jimg = nb // nj                  # images per chunk (32)
              imts = {}

              def load_imt(hc):
                  imt = impool.tile([128, 2 * rowlen], BF16, tag="im")
                  nc.sync.dma_start(imt[:], x9h[hc])
                  imts[hc] = imt

              # first image-row DMAs precede the warmup so conv1 matmuls
              # chain straight on and the PE clock stays at 2.4GHz
              load_imt(0)
              load_imt(1)

              with tc.tile_pool(name="c1ps", bufs=2,
                                space="PSUM") as pspool:
                psc0 = pspool.tile([128, 2048], F32, tag="ps")
                # HAM warmup into the first conv1 PSUM tile: no pool
                # boundary, so conv1's first matmul chains with no gap
                for _w in range(28):
                    nc.tensor.matmul(psc0[:, 0:512], dw_t[:, 0:128],
                                     dw_t[:, 0:512],
                                     start=(_w == 0), stop=(_w == 27))
                for hc in range(nh2):
                  imt = imts.pop(hc)
                  imv = imt[:].rearrange(
                      "p (h b w2 dx) -> p h b w2 dx",
                      h=2, b=nb, w2=HP // 2)
                  for j in range(nj):
                      # all 4 pool quadrants in one 4-bank PSUM tile
                      psc = psc0 if (hc == 0 and j == 0) else \
                          pspool.tile([128, 2048], F32, tag="ps")
                      for q, (dy, dx) in enumerate(
                              ((0, 0), (0, 1), (1, 0), (1, 1))):
                          nc.tensor.matmul(
                              psc[:, q * 512:(q + 1) * 512], w1_t[:],
                              imv[:, dy, j * jimg:(j + 1) * jimg,
                                  0:16, dx],
                              start=True, stop=True)
                      # eviction: one sign op, then the 4-way pool max
                      sq = sqpool.tile([128, 2048], BF16, tag="sq")
                      if (hc * nj + j) % 6 == 5:
                          nc.vector.tensor_scalar(sq[:], psc[:],
                                                  nb1_t[:], 2.0,
                                                  ALU.is_ge, ALU.mult)
                          nc.vector.tensor_scalar(sq[:], sq[:], 1.0,
                                                  None, ALU.subtract)
                      else:
                          nc.scalar.activation(sq[:], psc[:], AF.Sign,
                                               bias=b1_t[:])
                      m2 = m2pool.tile([128, 1024], BF16, tag="m2")
                      nc.vector.tensor_max(m2[:], sq[:, 0:1024],
                                           sq[:, 1024:2048])
                      m2v = m2[:].rearrange("p (d b x) -> p d b x",
                                            d=2, b=jimg)
                      nc.vector.tensor_max(
                          h1v[:, j * jimg:(j + 1) * jimg,
                              hc + 1, 1:P1PAD - 1],
                          m2v[:, 0], m2v[:, 1])
                  if hc + 2 < nh2:
                      load_imt(hc + 2)

          # ---- conv2 first image-half ----
          emit_conv2_half(tc, 0)

        with nc.semaphore(f"cc0_{_rep}") as cc0:
          # fire the first gather (no wait)
          with nc.Block() as blk:
              if ncores > 1:
                  @blk.gpsimd
                  def _(gp):
                      gp.collective_compute(
                          "AllGather", ALU.bypass,
                          replica_groups=[list(range(ncores))],
                          ins=[h2_shard[0]], outs=[h2_all[0]],
                      ).then_inc(cc0)
              else:
                  @blk.gpsimd
                  def _(gp):
                      gp.dma_start(h2_all[0, 0],
                                   h2_shard[0]).then_inc(cc0, 16)
          nc.all_engine_barrier()

          # ---- conv2 second image-half (gather 0 in flight) ----
          with tile.TileContext(nc) as tcb:
              emit_conv2_half(tcb, 1)

          with nc.semaphore(f"cc1_{_rep}") as cc1:
            # fire the second gather; wait only for the first
            with nc.Block() as blk:
                if ncores > 1:
                    @blk.gpsimd
                    def _(gp):
                        gp.collective_compute(
                            "AllGather", ALU.bypass,
                            replica_groups=[list(range(ncores))],
                            ins=[h2_shard[1]], outs=[h2_all[1]],
                        ).then_inc(cc1)
                        gp.wait_ge(cc0, 1)

                else:
                    @blk.gpsimd
                    def _(gp):
                        gp.dma_start(h2_all[1, 0],
                                     h2_shard[1]).then_inc(cc1, 16)
                        gp.wait_ge(cc0, 16)
            nc.all_engine_barrier()

            cctx.close()   # free conv-phase SBUF for the fc tiles

            # ---- fc over half 0 (second gather still in flight) ----
            def emit_fc_half(tc2, hf):
              with ExitStack() as ctx2:
                hgp = ctx2.enter_context(
                    tc2.tile_pool(name=f"hg{hf}", bufs=1))
                wp = ctx2.enter_context(
                    tc2.tile_pool(name=f"wfc{hf}", bufs=8))
                sp = ctx2.enter_context(
                    tc2.tile_pool(name=f"fc1out{hf}", bufs=1))
                psp = ctx2.enter_context(
                    tc2.tile_pool(name=f"fcps{hf}", bufs=2, space="PSUM"))
                p10 = ctx2.enter_context(
                    tc2.tile_pool(name=f"fc2ps{hf}", bufs=1, space="PSUM"))

                fc1b_t = sp.tile([FPC, 1], F32)
                nc.scalar.dma_start(fc1b_t[:], fc1b[:])
                f2hi_t = sp.tile([FPC, 10], FP16)
                nc.scalar.dma_start(f2hi_t[:], f2hi[:])
                f2lo_t = sp.tile([FPC, 10], FP16)
                nc.scalar.dma_start(f2lo_t[:], f2lo[:])

                # hg[ct]: all shards of this image half, channel-tile ct
                hg = {}
                for ct in range(2):
                    t = hgp.tile([128, nsh * nbh], FP8,
                                 tag=f"hg{ct}", name=f"hg{hf}{ct}")
                    for sh in range(nsh):
                        nc.scalar.dma_start(
                            t[:, sh * nbh:(sh + 1) * nbh],
                            h2_all[hf, sh, ct])
                    hg[ct] = t

                # hi and lo accumulate into separate PSUM banks so
                # consecutive matmuls alternate banks; summed at sign time
                psf = [psp.tile([128, bc_n], F32, tag=f"psf{i}",
                                name=f"psf{hf}{i}") for i in range(2)]
                for kt in range(NKT):
                    ct, s0 = kt // 64, kt % 64
                    wlot = wp.tile([128, FPC], FP16, tag="w")
                    nc.sync.dma_start(wlot[:], wlo[kt])
                    rhs = hg[ct][:].rearrange(
                        "p (sh s bh) -> p sh s bh",
                        sh=nsh, s=64)[:, :, s0, :]
                    nc.tensor.matmul(psf[0][:],
                                     whi_ap[:, kt * FPC:(kt + 1) * FPC],
                                     rhs, start=(kt == 0),
                                     stop=(kt == NKT - 1))
                    nc.tensor.matmul(psf[1][:], wlot[:], rhs,
                                     start=(kt == 0),
                                     stop=(kt == NKT - 1))

                ssum = sp.tile([128, bc_n], F32)
                nc.vector.tensor_copy(ssum[:], psf[0][:])
                nc.vector.tensor_add(ssum[:], ssum[:], psf[1][:])
                s1 = sp.tile([128, bc_n], FP16)
                nc.scalar.activation(s1[:], ssum[:], AF.Sign,
                                     bias=fc1b_t[:])
                ps10 = p10.tile([10, bc_n], F32, tag="ps10")
                nc.tensor.matmul(ps10[:], f2hi_t[:], s1[:],
                                 start=True, stop=False)
                nc.tensor.matmul(ps10[:], f2lo_t[:], s1[:],
                                 start=False, stop=True)
                yt = sp.tile([10, bc_n], F32)
                nc.scalar.copy(yt[:], ps10[:])
                # y columns: global image index sh*nb + hf*(nb/2) + bh
                yv = y_out.rearrange("o (sh hf bh) -> o sh hf bh",
                                     sh=nsh, hf=2)
                nc.scalar.dma_start(yv[:, :, hf, :],
                                    yt[:].rearrange(
                                        "o (sh bh) -> o sh bh", sh=nsh))

            with tile.TileContext(nc) as tc2:
                emit_fc_half(tc2, 0)

            # wait for the second gather
            with nc.Block() as blk:
                @blk.gpsimd
                def _(gp):
                    gp.wait_ge(cc1, 1 if ncores > 1 else 16)
            nc.all_engine_barrier()

            # ---- fc over half 1 ----
            with tile.TileContext(nc) as tc3:
                emit_fc_half(tc3, 1)

      nc.compile()
    return nc


_CACHE = {}


def _get_program(ncores=NCORES, nb=B):
    key = (ncores, nb)
    if key not in _CACHE:
        _CACHE[key] = build_program(ncores, nb)
    return _CACHE[key]


def _assemble(results, fc2_b):
    y = np.zeros((10, NB_ALL), np.float32)
    for n in range(NCORES):
        y += results[n]["y"]
    return (y.T + np.asarray(fc2_b, np.float32)[None, :]).astype(np.float32)


def kernel(**inputs):
    per_core = _host_prep(**inputs)
    nc = _get_program()
    res = run_bass_kernel_spmd(nc, per_core, core_ids=list(range(NCORES)))
    return _assemble(res.results, inputs["fc2_b"])


# revision 35
# speedup vs baseline: 1.0584x; 1.0584x over previous
"""Trainium2 Bass kernel for nn_BinaryConnectNet (binary CNN, 8 NeuronCores).

Sharding: batch-parallel convs (128 img/core), fc1 output-feature-sharded
(128 features/core) with an on-device AllGather of the binary activations
(fp8, values +-1). fc2 computed as per-core partials, summed on host.

Numerics:
 - conv1 (dw3x3 + 1x1 fused into a dense 3x3 with +-1 weights): input x is
   triple-bf16-split (exact to 2^-24 -- conv1 sign flips cascade through the
   binary net, so conv1 must be exact) contracted in one K=81 matmul per
   512-col quadrant chunk. All 4 pool quadrants accumulate into one
   [128,2048] PSUM tile; eviction is 3 ops: ACT sign(+bias) -> DVE dy-max ->
   GPSIMD dx-max writing +-1 fp8 into the padded h1 buffer.
 - conv2 depthwise: 9 PSUM-accumulated diagonal fp8 matmuls over shifted
   windows of h1. dw bias is folded into the pw bias (no nonlinearity
   between them), so dw eviction is a plain copy (odd ints |v|<=9, exact
   in fp8).
 - conv2 pointwise: dense K=128 fp8 matmul; sign bias = sgn(b_pw) +
   s_pw @ sgn(b_dw) (odd, so the pre-sign value is never 0).
 - fc1: weights split hi/lo fp16 (exact to 2^-22) accumulated into one PSUM;
   rhs is the gathered +-1 activations in fp8. The hi half is prefetched
   into a persistent SBUF tensor during the AllGather.
 - fc2: fp16 hi/lo, per-core partial output in fp32.
"""

import sys

for _p in ("/opt/trn_rl_repo",):
    if _p not in sys.path:
        sys.path.insert(0, _p)

import numpy as np
import ml_dtypes
from contextlib import ExitStack

import concourse.bass as bass
import concourse.bacc as bacc
import concourse.mybir as mybir
import concourse.tile as tile
from concourse.bass_utils import run_bass_kernel_spmd

F32 = mybir.dt.float32
BF16 = mybir.dt.bfloat16
FP16 = mybir.dt.float16
FP8 = mybir.dt.float8e4
AF = mybir.ActivationFunctionType
ALU = mybir.AluOpType

NCORES = 8
C1_SWDGE_POOL = False  # conv1 pooling via gpsimd accum-DMA (else DVE maxes)
B = 128               # images per core
H = 32                # conv1 spatial
HP = 34               # padded
ROWLEN = B * HP       # 4352: one padded h-row across batch (b, w) flattened
X9_SLACK = 8
X9_ROW = HP * ROWLEN + X9_SLACK   # flattened (h, b, w) per (c, s) row + slack
P1 = 16               # pooled spatial after pool1
P1PAD = 18
P2 = 8                # pooled spatial after pool2
NF1 = 1024            # fc1 features (global)
FPC = NF1 // NCORES   # fc1 features per core = 128
KFC = 256 * P2 * P2   # fc1 contraction = 16384
NKT = KFC // 128      # 128 K-tiles
NB_ALL = NCORES * B   # 1024


def _bf16(a):
    return np.asarray(a, dtype=ml_dtypes.bfloat16)


def _host_prep(x, w1_dw, b1_dw, w1_pw, b1_pw, w2_dw, b2_dw, w2_pw, b2_pw,
               fc1_w, fc1_b, fc2_w, fc2_b, ncores=NCORES, nb=B):
    """Build all per-core device input arrays (numpy only)."""
    sgn = np.sign
    x = np.asarray(x, np.float32)
    rowlen = nb * HP
    x9row = HP * rowlen + X9_SLACK
    fpc = FPC

    # triple bf16 split of x (exact to 2^-24)
    x0 = _bf16(x)
    r1 = x - x0.astype(np.float32)
    x1 = _bf16(r1)
    r2 = r1 - x1.astype(np.float32)
    x2 = _bf16(r2)
    splits = [x0, x1, x2]

    # x9h: per core [9 rows (3c+s), (h, b, w) flattened] bf16, pad 1
    x9h = np.zeros((ncores, 9, x9row), dtype=ml_dtypes.bfloat16)
    for s in range(3):
        xs = splits[s].reshape(ncores, nb, 3, H, H)
        for c in range(3):
            row = np.zeros((ncores, HP, nb, HP), dtype=ml_dtypes.bfloat16)
            row[:, 1:33, :, 1:33] = xs[:, :, c].transpose(0, 2, 1, 3)
            x9h[:, 3 * c + s, : HP * rowlen] = row.reshape(ncores, -1)
    # x81: host-side im2col so each pooled row needs ONE contiguous DMA:
    # [nh2, 81 rows (du,dv,c,s), 2 h-rows (h, b, w)]
    nh2_ = H // 2
    x81 = np.zeros((ncores, nh2_, 128, 2 * rowlen), dtype=ml_dtypes.bfloat16)
    for hc in range(nh2_):
        for du in range(3):
            for dv in range(3):
                r0 = 9 * (3 * du + dv)
                off = (2 * hc + du) * rowlen + dv
                x81[:, hc, r0:r0 + 9, :] = x9h[:, :, off:off + 2 * rowlen]
    # pad K from 81 to 128 (the PE clock monitor ignores partial-K matmuls,
    # leaving the array throttled to 1.2GHz): rows 81..127 duplicate rows
    # 0..46 and both copies carry half weight (+-0.5, exact in bf16)
    x81[:, :, 81:128, :] = x81[:, :, 0:47, :]

    # conv1 fused weights: lhsT [81, 128], rows (du,dv,c,s)
    s1dw = sgn(np.asarray(w1_dw, np.float32))[:, 0]       # [3, 3, 3]
    s1pw = sgn(np.asarray(w1_pw, np.float32))[:, :, 0, 0]  # [128, 3]
    w1t = np.zeros((128, 128), dtype=ml_dtypes.bfloat16)
    for du in range(3):
        for dv in range(3):
            for c in range(3):
                for s in range(3):
                    w1t[9 * (3 * du + dv) + 3 * c + s] = _bf16(
                        s1pw[:, c] * s1dw[c, du, dv])
    w1t[81:128] = w1t[0:47]
    w1t[0:47] = w1t[0:47] * _bf16(0.5)
    w1t[81:128] = w1t[81:128] * _bf16(0.5)
    b1eff = (sgn(np.asarray(b1_pw, np.float32))
             + s1pw @ sgn(np.asarray(b1_dw, np.float32))).astype(np.float32)

    # conv2 depthwise: 9 diagonal lhsT [128, 9*128] fp8
    s2dw = sgn(np.asarray(w2_dw, np.float32))[:, 0]       # [128, 3, 3]
    dwt = np.zeros((128, 9 * 128), dtype=ml_dtypes.bfloat16)
    for du in range(3):
        for dv in range(3):
            t = 3 * du + dv
            np.fill_diagonal(dwt[:, 128 * t:128 * (t + 1)],
                             _bf16(s2dw[:, du, dv]))
    dwb = sgn(np.asarray(b2_dw, np.float32))               # [128]

    # conv2 pointwise lhsT [128, 256] fp8; bias folds in the dw bias
    s2pw = sgn(np.asarray(w2_pw, np.float32))[:, :, 0, 0]  # [256, 128]
    pwt = _bf16(s2pw.T)                                    # [128, 256]
    b2m = (sgn(np.asarray(b2_pw, np.float32)) + s2pw @ dwb).astype(np.float32)

    # fc1 hi/lo fp16, column-permuted to device K-tile order, per-core slice
    fc1_w = np.asarray(fc1_w, np.float32)                  # [1024, 16384]
    # device feature order: kt = ct*64 + s0, partition c' -> col (ct*128+c')*64+s0
    cols = np.empty(KFC, np.int64)
    i = 0
    for ct in range(2):
        for s0 in range(64):
            for cp in range(128):
                cols[i] = (ct * 128 + cp) * 64 + s0
                i += 1
    wperm = fc1_w[:, cols]                                 # [1024, 16384(dev)]
    whi = wperm.astype(np.float16)
    wlo = (wperm - whi.astype(np.float32)).astype(np.float16)
    # whi: per-core [128(c'), NKT*128(kt-major)] for the one-shot SBUF DMA
    whi_t = whi.reshape(NCORES, fpc, NKT, 128).transpose(0, 3, 2, 1) \
        .reshape(NCORES, 128, NKT * fpc).copy()
    # wlo: per-core [NKT, 128(c'), 128(o_local)] streamed per kt
    wlo_t = wlo.reshape(NCORES, fpc, NKT, 128).transpose(0, 2, 3, 1).copy()
    b1fc = np.asarray(fc1_b, np.float32).reshape(NCORES, fpc, 1)

    # fc2 hi/lo fp16 per-core slice: lhsT [128(f_local), 10]
    fc2_w = np.asarray(fc2_w, np.float32)                  # [10, 1024]
    f2 = fc2_w.T.reshape(NCORES, fpc, 10)
    f2hi = f2.astype(np.float16)
    f2lo = (f2 - f2hi.astype(np.float32)).astype(np.float16)

    shared = {
        "w1t": w1t, "b1eff": b1eff.reshape(128, 1),
        "negb1": (-b1eff).reshape(128, 1).astype(np.float32),
        "dwt": dwt,
        "pwt": pwt, "b2m": b2m.reshape(2, 128).T.copy().astype(np.float32),
        "negb2": (-b2m).reshape(2, 128).T.copy().astype(np.float32),
    }
    per_core = []
    for n in range(ncores):
        d = dict(shared)
        d["x9h"] = x81[n]
        d["whi"] = whi_t[n]
        d["wlo"] = wlo_t[n]
        d["fc1b"] = b1fc[n]
        d["f2hi"] = f2hi[n]
        d["f2lo"] = f2lo[n]
        per_core.append(d)
    return per_core


def build_program(ncores=NCORES, nb=B, repeats=1):
    """Build the Bass program. nb = images per core. Returns nc."""
    rowlen = nb * HP
    x9row = HP * rowlen + X9_SLACK
    nsh = ncores                    # shards gathered for fc1
    nball = ncores * nb             # total batch
    nbc = 2                         # fc1 batch chunks = gather halves
    bc_n = nball // nbc             # fc1 chunk width (N per matmul)

    nc = bacc.Bacc("TRN2", target_bir_lowering=False, debug=False,
                   num_devices=ncores)

    def din(name, shape, dt):
        return nc.dram_tensor(name, shape, dt, kind="ExternalInput").ap()

    x9h = din("x9h", [H // 2, 128, 2 * rowlen], BF16)
    w1t = din("w1t", [128, 128], BF16)
    b1eff = din("b1eff", [128, 1], F32)
    negb1 = din("negb1", [128, 1], F32)
    dwt = din("dwt", [128, 9 * 128], BF16)
    pwt = din("pwt", [128, 256], BF16)
    b2m = din("b2m", [128, 2], F32)
    negb2 = din("negb2", [128, 2], F32)
    whi = din("whi", [128, NKT * FPC], FP16)
    wlo = din("wlo", [NKT, 128, FPC], FP16)
    fc1b = din("fc1b", [FPC, 1], F32)
    f2hi = din("f2hi", [FPC, 10], FP16)
    f2lo = din("f2lo", [FPC, 10], FP16)
    y_out = nc.dram_tensor("y", [10, nball], F32, kind="ExternalOutput").ap()

    # collective bounce buffers (fp8 +-1 activations), split in two
    # image-halves so the first gather overlaps conv2's second half
    nbh = nb * 32                   # elements per (half, mt): 64 s x nb/2
    h2_shard = nc.dram_tensor("h2_shard", [2, 2, 128, nbh], FP8).ap()
    h2_all = nc.dram_tensor("h2_all", [2, nsh, 2, 128, nbh], FP8,
                            addr_space="Shared").ap()

    nh2 = H // 2  # 16 pooled rows after pool1

    with ExitStack() as octx:
      # persistent SBUF living across the TileContext phases
      whi_sb = octx.enter_context(
          nc.sbuf_tensor("whi_sb", [128, NKT * FPC], FP16))
      whi_ap = whi_sb.ap()
      cctx = octx.enter_context(ExitStack())  # conv-phase persistents
      h1_sb = cctx.enter_context(
          nc.sbuf_tensor("h1_sb", [128, nb * P1PAD * P1PAD], FP8))
      h2_sb = [cctx.enter_context(
          nc.sbuf_tensor(f"h2_sb{m}", [128, nb * 64], FP8))
          for m in range(2)]
      dw_sb = cctx.enter_context(
          nc.sbuf_tensor("dw_sb", [128, 9 * 128], BF16))
      pw_sb = cctx.enter_context(nc.sbuf_tensor("pw_sb", [128, 256], BF16))
      b2_sb = cctx.enter_context(nc.sbuf_tensor("b2_sb", [128, 2], F32))
      nb2_sb = cctx.enter_context(nc.sbuf_tensor("nb2_sb", [128, 2], F32))
      dw_t, pw_t = dw_sb.ap(), pw_sb.ap()
      b2_t, nb2_t = b2_sb.ap(), nb2_sb.ap()
      h1v = h1_sb.ap().rearrange("p (b y x) -> p b y x", b=nb, y=P1PAD)

      cimg = 16                     # conv2 images per chunk
      ncch = nb // cimg
      ghalf = max(1, ncch // 2)     # g-chunks per gather half
      jw = 2                        # images per dw matmul
      jw2 = 8                       # images per pw matmul

      def emit_conv2_half(tc, hf):
        """Emit conv2 for image-half hf and ship its shard slices."""
        with tc.tile_pool(name=f"dwch{hf}", bufs=2) as dwpool, \
             tc.tile_pool(name=f"pwsq{hf}", bufs=5) as sq2pool, \
             tc.tile_pool(name=f"pwtr{hf}", bufs=2) as tr2pool, \
             tc.tile_pool(name=f"dwps{hf}", bufs=2, space="PSUM") as dps, \
             tc.tile_pool(name=f"pwps{hf}", bufs=2, space="PSUM") as pps:
            for gl in range(ghalf):
                g = hf * ghalf + gl
                dwc = dwpool.tile([128, cimg * 256], BF16, tag="dwc")
                gsub = 4             # images per dw psum
                for sub in range(cimg // gsub):
                    ps = dps.tile([128, gsub * 256], F32, tag="dps")
                    for t in range(9):
                        du, dv = t // 3, t % 3
                        for j in range(gsub // jw):
                            b0 = g * cimg + sub * gsub + j * jw
                            nc.tensor.matmul(
                                ps[:, j * jw * 256:(j + 1) * jw * 256],
                                dw_t[:, 128 * t:128 * (t + 1)],
                                h1v[:, b0:b0 + jw, du:du + P1,
                                    dv:dv + P1],
                                start=(t == 0), stop=(t == 8))
                    # dw bias folded into pw bias: plain copy eviction
                    if sub % 2 == 0:
                        nc.scalar.copy(
                            dwc[:, sub * gsub * 256:(sub + 1) * gsub * 256],
                            ps[:])
                    else:
                        nc.vector.tensor_copy(
                            dwc[:, sub * gsub * 256:(sub + 1) * gsub * 256],
                            ps[:])
                dwv = dwc[:].rearrange(
                    "p (b y2 dy x2 dx) -> p b y2 dy x2 dx",
                    b=cimg, y2=P2, dy=2, x2=P2)
                for mt in range(2):
                    sq2 = []
                    for qi, (dy, dx) in enumerate(
                            ((0, 0), (0, 1), (1, 0), (1, 1))):
                        sqt = sq2pool.tile([128, cimg * 64], BF16,
                                           tag="sq2")
                        ps = pps.tile([128, cimg * 64], F32, tag="pps")
                        for j in range(cimg // jw2):
                            nc.tensor.matmul(
                                ps[:, j * jw2 * 64:(j + 1) * jw2 * 64],
                                pw_t[:, 128 * mt:128 * (mt + 1)],
                                dwv[:, j * jw2:(j + 1) * jw2,
                                    :, dy, :, dx],
                                start=True, stop=True)
                        if qi < 3:
                            nc.scalar.activation(
                                sqt[:], ps[:], AF.Sign,
                                bias=b2_t[:, mt:mt + 1])
                        else:
                            nc.vector.tensor_scalar(
                                sqt[:], ps[:], nb2_t[:, mt:mt + 1],
                                2.0, ALU.is_ge, ALU.mult)
                            nc.vector.tensor_scalar(
                                sqt[:], sqt[:], 1.0, None, ALU.subtract)
                        sq2.append(sqt)
                    u01 = tr2pool.tile([128, cimg * 64], BF16, tag="tr2")
                    u23 = tr2pool.tile([128, cimg * 64], BF16, tag="tr2")
                    nc.vector.tensor_max(u01[:], sq2[0][:], sq2[1][:])
                    nc.vector.tensor_max(u23[:], sq2[2][:], sq2[3][:])
                    # h2 layout: (half, bh, s) so shard halves are contiguous
                    h2v = h2_sb[mt].ap().rearrange(
                        "p (hf s bh) -> p hf bh s", hf=2, s=64)
                    bh0 = gl * cimg
                    nc.vector.tensor_max(
                        h2v[:, hf, bh0:bh0 + cimg, :],
                        u01[:].rearrange("p (b s) -> p b s", b=cimg),
                        u23[:].rearrange("p (b s) -> p b s", b=cimg))
        # ship this half of the shard (contiguous 16B-aligned slices)
        for mt in range(2):
            nc.scalar.dma_start(h2_shard[hf, mt],
                                h2_sb[mt].ap()[:, hf * nbh:(hf + 1) * nbh])

      for _rep in range(repeats):
        with tile.TileContext(nc) as tc, ExitStack() as ctx:
          cpool = ctx.enter_context(tc.tile_pool(name="consts", bufs=1))
          w1_t = cpool.tile([128, 128], BF16)
          nc.scalar.dma_start(w1_t[:], w1t[:])
          b1_t = cpool.tile([128, 1], F32)
          nc.scalar.dma_start(b1_t[:], b1eff[:])
          nb1_t = cpool.tile([128, 1], F32)
          nc.scalar.dma_start(nb1_t[:], negb1[:])
          nc.scalar.dma_start(dw_t[:], dwt[:])
          nc.scalar.dma_start(pw_t[:], pwt[:])
          nc.scalar.dma_start(b2_t[:], b2m[:])
          nc.scalar.dma_start(nb2_t[:], negb2[:])
          # prefetch the fc1 hi weights during conv (scalar DMA queue)
          nc.scalar.dma_start(whi_ap[:], whi[:])

          # zero the h1 pad border
          nc.vector.memset(h1v[:, :, 0, :], 0.0)
          nc.vector.memset(h1v[:, :, P1PAD - 1, :], 0.0)
          nc.vector.memset(h1v[:, :, 1:P1PAD - 1, 0], 0.0)
          nc.vector.memset(h1v[:, :, 1:P1PAD - 1, P1PAD - 1], 0.0)

          # ---- conv1 + pool1 -> h1 (padded, +-1 fp8) ----
          with tc.tile_pool(name="c1work", bufs=3) as impool, \
               tc.tile_pool(name="c1sq", bufs=4) as sqpool, \
               tc.tile_pool(name="c1m2", bufs=4) as m2pool:
              nj = (nb * 16) // 512            # 512-col chunks per hc
              jimg = nb // nj                  # images per chunk (32)
              imts = {}

              def load_imt(hc):
                  imt = impool.tile([128, 2 * rowlen], BF16, tag="im")
                  nc.sync.dma_start(imt[:], x9h[hc])
                  imts[hc] = imt

              # first image-row DMAs precede the warmup so conv1 matmuls
              # chain straight on and the PE clock stays at 2.4GHz
              load_imt(0)
              load_imt(1)

              with tc.tile_pool(name="c1ps", bufs=2,
                                space="PSUM") as pspool:
                psc0 = pspool.tile([128, 2048], F32, tag="ps")
                # HAM warmup into the first conv1 PSUM tile: no pool
                # boundary, so conv1's first matmul chains with no gap
                for _w in range(28):
                    nc.tensor.matmul(psc0[:, 0:512], dw_t[:, 0:128],
                                     dw_t[:, 0:512],
                                     start=(_w == 0), stop=(_w == 27))
                for hc in range(nh2):
                  imt = imts.pop(hc)
                  imv = imt[:].rearrange(
                      "p (h b w2 dx) -> p h b w2 dx",
                      h=2, b=nb, w2=HP // 2)
                  for j in range(nj):
                      # all 4 pool quadrants in one 4-bank PSUM tile
                      psc = psc0 if (hc == 0 and j == 0) else \
                          pspool.tile([128, 2048], F32, tag="ps")
                      for q, (dy, dx) in enumerate(
                              ((0, 0), (0, 1), (1, 0), (1, 1))):
                          nc.tensor.matmul(
                              psc[:, q * 512:(q + 1) * 512], w1_t[:],
                              imv[:, dy, j * jimg:(j + 1) * jimg,
                                  0:16, dx],
                              start=True, stop=True)
                      # eviction: one sign op, then the 4-way pool max
                      sq = sqpool.tile([128, 2048], BF16, tag="sq")
                      if (hc * nj + j) % 6 == 5:
                          nc.vector.tensor_scalar(sq[:], psc[:],
                                                  nb1_t[:], 2.0,
                                                  ALU.is_ge, ALU.mult)
                          nc.vector.tensor_scalar(sq[:], sq[:], 1.0,
                                                  None, ALU.subtract)
                      else:
                          nc.scalar.activation(sq[:], psc[:], AF.Sign,
                                               bias=b1_t[:])
                      m2 = m2pool.tile([128, 1024], BF16, tag="m2")
                      nc.vector.tensor_max(m2[:], sq[:, 0:1024],
                                           sq[:, 1024:2048])
                      m2v = m2[:].rearrange("p (d b x) -> p d b x",
                                            d=2, b=jimg)
                      nc.vector.tensor_max(
                          h1v[:, j * jimg:(j + 1) * jimg,
                              hc + 1, 1:P1PAD - 1],
                          m2v[:, 0], m2v[:, 1])
                  if hc + 2 < nh2:
                      load_imt(hc + 2)

          # ---- conv2 first image-half ----
          emit_conv2_half(tc, 0)

        with nc.semaphore(f"cc0_{_rep}") as cc0, \
             nc.semaphore(f"cc1_{_rep}") as cc1:
          # fire the first gather (no wait)
          with nc.Block() as blk:
              if ncores > 1:
                  @blk.gpsimd
                  def _(gp):
                      gp.collective_compute(
                          "AllGather", ALU.bypass,
                          replica_groups=[list(range(ncores))],
                          ins=[h2_shard[0]], outs=[h2_all[0]],
                      ).then_inc(cc0)
              else:
                  @blk.gpsimd
                  def _(gp):
                      gp.dma_start(h2_all[0, 0],
                                   h2_shard[0]).then_inc(cc0, 16)
          nc.all_engine_barrier()

          # ---- conv2 second image-half (gather 0 in flight) ----
          with tile.TileContext(nc) as tcb:
              emit_conv2_half(tcb, 1)

          # fire the second gather; wait only for the first
          with nc.Block() as blk:
              if ncores > 1:
                  @blk.gpsimd
                  def _(gp):
                      gp.collective_compute(
                          "AllGather", ALU.bypass,
                          replica_groups=[list(range(ncores))],
                          ins=[h2_shard[1]], outs=[h2_all[1]],
                      ).then_inc(cc1)
                      gp.wait_ge(cc0, 1 if ncores > 1 else 16)
              else:
                  @blk.gpsimd
                  def _(gp):
                      gp.dma_start(h2_all[1, 0],
                                   h2_shard[1]).then_inc(cc1, 16)
                      gp.wait_ge(cc0, 16)
          nc.all_engine_barrier()

          cctx.close()   # free conv-phase SBUF for the fc tiles

          # hg[(ct, hf)]: all shards of image-half hf, channel-tile ct
          # (persistent so the half-0 loads overlap the second gather)
          hg = {}
          with ExitStack() as hctx:
            for ct in range(2):
                for hf in range(2):
                    hg[(ct, hf)] = hctx.enter_context(
                        nc.sbuf_tensor(f"hg{ct}{hf}_{_rep}",
                                       [128, nsh * nbh], FP8))

            def load_hg(hf):
                for ct in range(2):
                    t = hg[(ct, hf)].ap()
                    for sh in range(nsh):
                        eng = nc.scalar if sh % 2 == 0 else nc.sync
                        eng.dma_start(t[:, sh * nbh:(sh + 1) * nbh],
                                      h2_all[hf, sh, ct])

            # load the half-0 activations while gather 1 runs
            with tile.TileContext(nc) as tcw:
                load_hg(0)

            with nc.Block() as blk:
                @blk.gpsimd
                def _(gp):
                    gp.wait_ge(cc1, 1 if ncores > 1 else 16)
            nc.all_engine_barrier()

            # ---- fc1 + fc2 ----
            with tile.TileContext(nc) as tc2, ExitStack() as ctx2:
                wp = ctx2.enter_context(tc2.tile_pool(name="wfc", bufs=8))
                sp = ctx2.enter_context(tc2.tile_pool(name="fc1out",
                                                      bufs=1))
                psp = ctx2.enter_context(
                    tc2.tile_pool(name="fcps", bufs=2, space="PSUM"))
                p10 = ctx2.enter_context(
                    tc2.tile_pool(name="fc2ps", bufs=2, space="PSUM"))

                load_hg(1)
                fc1b_t = sp.tile([FPC, 1], F32)
                nc.scalar.dma_start(fc1b_t[:], fc1b[:])
                f2hi_t = sp.tile([FPC, 10], FP16)
                nc.scalar.dma_start(f2hi_t[:], f2hi[:])
                f2lo_t = sp.tile([FPC, 10], FP16)
                nc.scalar.dma_start(f2lo_t[:], f2lo[:])

                s1 = sp.tile([128, nball], FP16)
                psf = [psp.tile([128, bc_n], F32, tag=f"psf{hf}",
                                name=f"psf{hf}") for hf in range(2)]
                yt = sp.tile([10, nball], F32)

                # kt-outer, halves interleaved: consecutive matmuls
                # alternate PSUM banks and each weight tile is reused
                for kt in range(NKT):
                    ct, s0 = kt // 64, kt % 64
                    wlot = wp.tile([128, FPC], FP16, tag="w")
                    nc.sync.dma_start(wlot[:], wlo[kt])
                    rhs = [hg[(ct, hf)].ap().rearrange(
                        "p (sh s bh) -> p sh s bh",
                        sh=nsh, s=64)[:, :, s0, :] for hf in range(2)]
                    for hf in range(2):
                        nc.tensor.matmul(
                            psf[hf][:],
                            whi_ap[:, kt * FPC:(kt + 1) * FPC],
                            rhs[hf], start=(kt == 0), stop=False)
                    for hf in range(2):
                        nc.tensor.matmul(psf[hf][:], wlot[:], rhs[hf],
                                         start=False,
                                         stop=(kt == NKT - 1))

                yv = y_out.rearrange("o (sh hf bh) -> o sh hf bh",
                                     sh=nsh, hf=2)
                for hf in range(2):
                    nc.scalar.activation(
                        s1[:, hf * bc_n:(hf + 1) * bc_n],
                        psf[hf][:], AF.Sign, bias=fc1b_t[:])
                    ps10 = p10.tile([10, bc_n], F32, tag="ps10")
                    nc.tensor.matmul(ps10[:], f2hi_t[:],
                                     s1[:, hf * bc_n:(hf + 1) * bc_n],
                                     start=True, stop=False)
                    nc.tensor.matmul(ps10[:], f2lo_t[:],
                                     s1[:, hf * bc_n:(hf + 1) * bc_n],
                                     start=False, stop=True)
                    yt_h = yt[:, hf * bc_n:(hf + 1) * bc_n]
                    nc.scalar.copy(yt_h, ps10[:])
                    nc.scalar.dma_start(
                        yv[:, :, hf, :],
                        yt_h.rearrange("o (sh bh) -> o sh bh", sh=nsh))

      nc.compile()
    return nc


_CACHE = {}


def _get_program(ncores=NCORES, nb=B):
    key = (ncores, nb)
    if key not in _CACHE:
        _CACHE[key] = build_program(ncores, nb)
    return _CACHE[key]


def _assemble(results, fc2_b):
    y = np.zeros((10, NB_ALL), np.float32)
    for n in range(NCORES):
        y += results[n]["y"]
    return (y.T + np.asarray(fc2_b, np.float32)[None, :]).astype(np.float32)


def kernel(**inputs):
    per_core = _host_prep(**inputs)
    nc = _get_program()
    res = run_bass_kernel_spmd(nc, per_core, core_ids=list(range(NCORES)))
    return _assemble(res.results, inputs["fc2_b"])
